# revision 10
# baseline (speedup 1.0000x reference)
"""Trainium2 Bass kernel for nn_CMLITargetLoss (CMLI target loss), v2.3.

Data-parallel over batch: 64 samples -> 8 NeuronCores x 8 samples.
text/target ship bf16, image ships fp8-e4m3 (loss terms are large sums;
quantization bias ~1e-3 << the 2e-2 gate). Per-core partial sums return
in three small f32 tiles; host combines the 8 cores' scalars.

Per core (samples b=0..7, pairs p=b//2 at row slots (tile h, part 0/64)):
  r2 rows  = sum_d g^2 (squares on DVE, PE ones-matmul chains into 2 psum
             banks at partitions 0/64); gn=sqrt (ACT), rinv=1/gn bf16 (DVE)
  repS[p]  = rinv row broadcast to 128 partitions via stride-0 DMA (Pool)
  sim      = t[1:] @ g[1:]^T bf16; pairs 0-2 stream their 6 psum chunk-
             chains through the t phase (t3..t5 arrive early on the ACT
             DMA queue), pair 3 recycles pair 0's banks at the tail.
  posts    = ttr (DVE): scaled=sim*rep, M=rowmax fused; stt (Pool):
             Ac = sum((scaled>=M)*rep) = rinv_sel
  S1       = sum keep*(q^2 - 2*M*q), q=1/rinv_sel, + sum keep*||t||^2
             (tsq chains reuse the r2 psum banks; tsq matmuls run last)
  S2 (cls) = sum (t0-g0)^2 (Pool); S3 = sum i^2 - 2 sum i*g + sum g^2
"""

import os
import sys

import numpy as np

for _p in ("/opt/trn_rl_repo", "/root/.axon_site/_ro/trn_rl_repo"):
    if os.path.isdir(_p) and _p not in sys.path:
        sys.path.insert(0, _p)

B, T, D = 64, 197, 768
NC_ = 8            # cores
BL = B // NC_      # 8 local samples per core
NP = BL // 2       # 4 sample pairs
KD = D // 128      # 6 d-chunks
TM1 = T - 1        # 196
C0, C1 = 128, TM1 - 128   # t-chunk sizes 128, 68
MCOLS = 16 + 8 * T        # masks: kA0(8) kA1(8) krow[p] x4 (394 each)
ROWPOS = [(0, 0), (0, 64), (1, 0), (1, 64)]  # pair -> (row tile, partition)
KORD = [3, 4, 5, 0, 1, 2]  # sim chunk order (t3..t5 arrive first)
TQORD = [3, 4, 5, 0, 1, 2]  # tsq chain order (matches sqt emission)

OUT_NAMES = ["outA", "outB", "outC"]

_CACHE = {}

# ostA column map (early writers)
CA_I2 = 0      # +j (j=0..1)  sum i^2 over chunk pairs (k01, k23)
CA_IG = 5      # +k (k=0..3)  sum i*g
CA_CLS = 9     # +k (k=0..5)  cls (t0-g0)^2
CA_G2 = 15     # +h (h=0..1)  sum g^2 (valid at partitions 0,64)
# ostB column map (colmath tail)
CB_V2C0 = 0    # [0:128] sum keep*(q^2-2Mq) chunk c0
CB_V2C1 = 1    # [0:68]  chunk c1
CB_I2_5 = 2    # k=5 sum i^2
CB_I2_4 = 5    # k=4 sum i^2
CB_IG_4 = 3    # k=4 sum i*g
CB_IG_5 = 4    # k=5 sum i*g
# ostC column map (tsq tail)
CC_TSQ = 0     # +h (h=0..1)  sum keep*||t||^2 (valid at partitions 0,64)


def _build():
    import concourse.bacc as bacc
    import concourse.tile as tile
    from concourse import mybir
    from contextlib import ExitStack

    f32 = mybir.dt.float32
    f16 = mybir.dt.float16
    bf16 = mybir.dt.bfloat16
    fp8 = mybir.dt.float8e4
    Alu = mybir.AluOpType
    Act = mybir.ActivationFunctionType

    nc = bacc.Bacc("TRN2", target_bir_lowering=False, debug=False)

    tT = nc.dram_tensor("textT", (D, BL, T), bf16, kind="ExternalInput")
    gT = nc.dram_tensor("targetT", (D, BL, T), bf16, kind="ExternalInput")
    iT = nc.dram_tensor("imageT", (D, BL, T), bf16, kind="ExternalInput")
    masksD = nc.dram_tensor("masks", (128, MCOLS), bf16, kind="ExternalInput")
    outA_d = nc.dram_tensor("outA", (128, 24), f32, kind="ExternalOutput")
    outB_d = nc.dram_tensor("outB", (128, 8), f32, kind="ExternalOutput")
    outC_d = nc.dram_tensor("outC", (128, 8), f32, kind="ExternalOutput")

    with tile.TileContext(nc) as tc, ExitStack() as ctx:
        consts = ctx.enter_context(tc.tile_pool(name="consts", bufs=1))
        inputs = ctx.enter_context(tc.tile_pool(name="inputs", bufs=1))
        sbuf = ctx.enter_context(tc.tile_pool(name="sbuf", bufs=1))
        scratch = ctx.enter_context(tc.tile_pool(name="scratch", bufs=1))
        psum = ctx.enter_context(tc.tile_pool(name="psum", bufs=1, space="PSUM"))

        # ---------------- constants ----------------
        ones_f16 = consts.tile([128, 1], f16, tag="ones_f16")
        nc.vector.memset(ones_f16, 1.0)
        ones_bf = consts.tile([128, 128], bf16, tag="ones_bf")
        nc.vector.memset(ones_bf, 1.0)

        masks = consts.tile([128, MCOLS], bf16, tag="masks")
        kA0 = masks[:, 0:8]
        kA1 = masks[:68, 8:16]
        krow = [masks[:, 16:16 + 2 * T],
                masks[:, 16 + 2 * T:16 + 4 * T]]

        ostA = consts.tile([128, 24], f32, tag="ostA")
        nc.vector.memset(ostA, 0.0)
        ostB = consts.tile([128, 8], f32, tag="ostB")
        nc.vector.memset(ostB, 0.0)
        ostC = consts.tile([128, 8], f32, tag="ostC")
        nc.vector.memset(ostC, 0.0)

        # resident inputs
        t_all = inputs.tile([128, KD, BL, T], bf16, tag="t_all")
        g_all = inputs.tile([128, KD, BL, T], bf16, tag="g_all")
        i_all = inputs.tile([128, KD, BL, T], bf16, tag="i_all")

        def _ld(eng, dst, srcdram, k):
            eng.dma_start(
                out=dst[:, k].rearrange("p b t -> p (b t)"),
                in_=srcdram[k * 128:(k + 1) * 128].rearrange("p b t -> p (b t)"))

        # psum row tiles: r2 chains then (sequentially) tsq chains; unused
        # lanes memset to 1.0 so full-tile Sqrt/Reciprocal stay legal.
        rows = [psum.tile([128, 512], f32, tag=f"row{p}", name=f"rows_{p}")
                for p in range(NP)]

        # Pool DMA queue: image chunks 2,3
        for k in (2, 3):
            _ld(nc.gpsimd, i_all, iT, k)

        # ACT DMA queue: t3..t5 arrive in parallel with SP's g stream
        for k in (3, 4, 5):
            _ld(nc.scalar, t_all, tT, k)
        # trigger the act-table loads in ACT's idle window (they otherwise
        # attach to gn's sem waits and land on the rinv critical path)
        actwarm = consts.tile([1, 1], f32, tag="actwarm")
        nc.vector.memset(actwarm, 1.0)
        actwarm2 = consts.tile([1, 1], f32, tag="actwarm2")
        nc.scalar.activation(actwarm2, actwarm, Act.Sqrt)

        # SP DMA queue: g (first chunk split in half for an earlier start),
        # t0..t2, masks, i4, i5; output DMAs at the very end.
        HB = BL * T // 2
        g0d = gT[0:128].rearrange("p b t -> p (b t)")
        g0s = t_all  # placeholder to appease linters
        nc.sync.dma_start(out=g_all[:, 0].rearrange("p b t -> p (b t)")[:, 0:HB],
                          in_=g0d[:, 0:HB])
        nc.sync.dma_start(out=g_all[:, 0].rearrange("p b t -> p (b t)")[:, HB:],
                          in_=g0d[:, HB:])
        for k in range(1, KD):
            _ld(nc.sync, g_all, gT, k)
        for k in (0, 1, 2):
            _ld(nc.sync, t_all, tT, k)
        nc.sync.dma_start(out=masks, in_=masksD[:, :])
        for k in (0, 1, 4, 5):
            _ld(nc.sync, i_all, iT, k)

        # ---------------- PE ramp (8 cheap bf16 matmuls) ----------------
        warm_ps = psum.tile([128, 512], f32, tag="sim", name="warm", bufs=4)
        for w in range(8):
            nc.tensor.matmul(warm_ps[:, 0:128], ones_bf, ones_bf,
                             start=True, stop=True, skip_group_check=True)

        # ---------------- g squares (DVE) + r2 row chains (PE) ------------
        def _sqg(k):
            sq = scratch.tile([128, BL, T], f16, tag="sqg", bufs=3,
                              name=f"sqg_{k}")
            if k == 0:
                sqv = sq.rearrange("p b t -> p (b t)")
                gv = g_all[:, 0].rearrange("p b t -> p (b t)")
                nc.vector.tensor_mul(sqv[:, 0:HB], gv[:, 0:HB], gv[:, 0:HB])
                nc.vector.tensor_mul(sqv[:, HB:], gv[:, HB:], gv[:, HB:])
            elif k >= 3:
                nc.gpsimd.tensor_mul(sq, g_all[:, k], g_all[:, k])
            else:
                nc.vector.tensor_mul(sq, g_all[:, k], g_all[:, k])
            return sq

        def _r2k(sq, k):
            for p in range(NP):
                nc.tensor.matmul(
                    rows[p][0:1, 0:2 * T], ones_f16,
                    sq[:, 2 * p:2 * p + 2, :],
                    start=(k == 0), stop=(k == KD - 1),
                    skip_group_check=True,
                )

        sim_ps = {}

        def _sim_k(pairs, k, first, last):
            for p in pairs:
                for ci, (P, lo, hi) in enumerate(((C0, 1, 1 + C0),
                                                  (C1, 1 + C0, T))):
                    key = (p, ci)
                    if key not in sim_ps:
                        sim_ps[key] = psum.tile([128, 512], f32, tag="sim",
                                                name=f"ps_{p}_{ci}", bufs=4)
                    ps = sim_ps[key]
                    for s in range(2):
                        b = 2 * p + s
                        nc.tensor.matmul(
                            ps[:P, 256 * s:256 * s + TM1],
                            t_all[:, k, b, lo:hi], g_all[:, k, b, 1:T],
                            start=(first and s == 0),
                            stop=(last and s == 1),
                        )

        # PE order tracks data arrival: r2 chunks 0-2 (g stream), sims for
        # the early t3..t5, r2 chunks 3-5, then sims for t0..t2.
        for k in (0, 1, 2):
            _r2k(_sqg(k), k)
        for ki, k in enumerate((3, 4, 5)):
            _sim_k((0, 1, 2), k, first=(ki == 0), last=False)
        for k in (3, 4, 5):
            _r2k(_sqg(k), k)

        # gn = sqrt(r2) per pair row; rinv = 1/gn (bf16, partition 0)
        rinv4 = []
        for p in range(NP):
            gn = sbuf.tile([1, 2 * T], f32, tag=f"gn_{p}", name=f"gn_{p}")
            nc.scalar.activation(gn, rows[p][0:1, 0:2 * T], Act.Sqrt)
            rv = sbuf.tile([1, 2 * T], bf16, tag=f"rinv4_{p}",
                           name=f"rinv4_{p}")
            with nc.allow_low_precision(reason="rinv bf16 feeds bf16 scale"):
                nc.vector.reciprocal(rv, gn)
            rinv4.append(rv)

        for k in (0, 1, 2):
            _sim_k((0, 1, 2), k, first=False, last=(k == 2))

        # rinv replicated across partitions: PE outer products (base 0)
        # into each pair's own row bank, staged to SBUF f32 for the posts.
        rep_ps = {}
        for p in range(NP):
            rp = psum.tile([128, 512], f32, tag=f"row{p}",
                           name=f"rep_{p}", bufs=1)
            for s in range(2):
                nc.tensor.matmul(
                    rp[:, s * T:s * T + T], ones_bf[0:1, :],
                    rinv4[p][0:1, s * T:s * T + T])
            rsb = sbuf.tile([128, 2 * T], f32, tag=f"repsb_{p}",
                            name=f"repsb_{p}")
            nc.scalar.activation(rsb, rp[:, 0:2 * T], Act.Copy)
            rep_ps[p] = rsb

        # pair 3 sims on recycled sim banks
        for ki, k in enumerate(KORD):
            _sim_k((3,), k, first=(ki == 0), last=(ki == KD - 1))

        # ---------------- image terms ----------------
        # S3 = sum (i-g)^2 directly: Pool subtract + ACT square-accum
        # (the HW-proven baseline pattern). k0..3 -> ostA, k4/k5 -> ostB.
        d_acc = [ostA[:, CA_I2 + k:CA_I2 + k + 1] for k in range(4)]
        d_acc += [ostB[:, CB_I2_4:CB_I2_4 + 1], ostB[:, CB_I2_5:CB_I2_5 + 1]]
        for k in range(KD):
            dk = scratch.tile([128, BL, T], f16, tag="dimg", bufs=2,
                              name=f"dimg_{k}")
            nc.gpsimd.tensor_sub(dk, i_all[:, k], g_all[:, k])
            dsq = scratch.tile([128, BL, T], f16, tag="dsq", bufs=2,
                               name=f"dsq_{k}")
            nc.scalar.activation(dsq, dk, Act.Square, accum_out=d_acc[k])

        # cls diff + square-accum (Pool, paced by t arrivals)
        for k in KORD:
            d0 = scratch.tile([128, BL], f32, tag="cls", bufs=2,
                              name=f"cls_{k}")
            nc.gpsimd.tensor_sub(d0, t_all[:, k, :, 0], g_all[:, k, :, 0])
            d0s = scratch.tile([128, BL], f32, tag="cls", bufs=2,
                               name=f"clsq_{k}")
            nc.scalar.activation(d0s, d0, Act.Square,
                                 accum_out=ostA[:, CA_CLS + k:CA_CLS + k + 1])

        # ---------------- posts + interleaved t squares ----------------
        Mc = [sbuf.tile([128, BL], f32, tag=f"Mc{ci}", name=f"Mc{ci}")
              for ci in range(2)]
        Ac = [sbuf.tile([128, BL], f32, tag=f"Ac{ci}", name=f"Ac{ci}")
              for ci in range(2)]

        sqt = {}

        def _sqt(k, eng=None):
            st = scratch.tile([128, BL, T], f16, tag="sqt", bufs=3,
                              name=f"sqt_{k}")
            eng = eng or (nc.gpsimd if k in (3, 4, 5) else nc.vector)
            eng.tensor_mul(st, t_all[:, k], t_all[:, k])
            sqt[k] = st

        def _post(p):
            for ci, P in enumerate((C0, C1)):
                ps = sim_ps[(p, ci)]
                rp = rep_ps[p]
                for s in range(2):
                    b = 2 * p + s
                    ss = scratch.tile([128, TM1], f32, tag="ss", bufs=4,
                                      name=f"ss_{p}_{ci}_{s}")
                    nc.vector.tensor_mul(ss[:P], ps[:P, 256 * s:256 * s + TM1],
                                         rp[:P, s * T + 1:s * T + T])
                    nc.vector.reduce_max(Mc[ci][:P, b:b + 1], ss[:P],
                                         axis=mybir.AxisListType.X)
                    sj = scratch.tile([128, TM1], f32, tag="sj", bufs=4,
                                      name=f"sj_{p}_{ci}_{s}")
                    nc.vector.scalar_tensor_tensor(
                        out=sj[:P], in0=ss[:P], scalar=Mc[ci][:P, b:b + 1],
                        in1=rp[:P, s * T + 1:s * T + T],
                        op0=Alu.is_ge, op1=Alu.mult,
                        accum_out=Ac[ci][:P, b:b + 1],
                    )

        # DVE order: early t squares fill the pre-post window; sqt_2 runs
        # on Pool so the ttr chain owns the DVE tail.
        _sqt(3)
        _sqt(4)
        _post(0)
        _sqt(5)
        _post(1)
        _sqt(0)
        _post(2)
        _sqt(1)
        _sqt(2)
        _post(3)

        # tsq row chains (PE): recycled sim banks, one per pair, base 0
        tsqr = [psum.tile([128, 512], f32, tag="sim", name=f"tsqr_{p}",
                          bufs=4) for p in range(NP)]
        for ti, k in enumerate(TQORD):
            for p in range(NP):
                nc.tensor.matmul(
                    tsqr[p][0:1, 0:2 * T], ones_f16,
                    sqt[k][:, 2 * p:2 * p + 2, :],
                    start=(ti == 0), stop=(ti == KD - 1),
                    skip_group_check=True,
                )

        # masked sum of ||t||^2 rows (reads only the written psum row)
        for p in range(NP):
            trjunk = scratch.tile([1, 2 * T], f32, tag="trjunk", bufs=4,
                                  name=f"trjunk_{p}")
            nc.vector.scalar_tensor_tensor(
                out=trjunk, in0=tsqr[p][0:1, 0:2 * T], scalar=1.0,
                in1=masks[0:1, 16 + 2 * T * p:16 + 2 * T * (p + 1)],
                op0=Alu.mult, op1=Alu.mult,
                accum_out=ostC[0:1, CC_TSQ + p:CC_TSQ + p + 1],
            )

        # ---------------- column math ----------------
        # Ac holds rinv_sel; q = 1/rinv_sel = |g_sel|;
        # v2 = q^2 - 2*M*q  (M*q = raw dot at argmax)
        for ci, (P, kA, col) in enumerate(((C0, kA0, CB_V2C0),
                                           (C1, kA1, CB_V2C1))):
            q = scratch.tile([128, BL], f32, tag="colm", bufs=8,
                             name=f"q_{ci}")
            nc.vector.reciprocal(q[:P], Ac[ci][:P])
            a2 = scratch.tile([128, BL], f32, tag="colm", bufs=8,
                              name=f"a2_{ci}")
            nc.vector.scalar_tensor_tensor(
                out=a2[:P], in0=Mc[ci][:P], scalar=-2.0, in1=q[:P],
                op0=Alu.mult, op1=Alu.mult)
            v2 = scratch.tile([128, BL], f32, tag="colm", bufs=8,
                              name=f"v2_{ci}")
            nc.gpsimd.tensor_mul(v2[:P], q[:P], q[:P])
            v2k = scratch.tile([128, BL], f32, tag="colm", bufs=8,
                               name=f"v2k_{ci}")
            nc.gpsimd.tensor_add(v2k[:P], v2[:P], a2[:P])
            v2m = scratch.tile([128, BL], f32, tag="colm", bufs=8,
                               name=f"v2m_{ci}")
            nc.vector.scalar_tensor_tensor(
                out=v2m[:P], in0=v2k[:P], scalar=1.0, in1=kA,
                op0=Alu.mult, op1=Alu.mult,
                accum_out=ostB[:P, col:col + 1],
            )

        nc.sync.dma_start(out=outA_d[:, :], in_=ostA)
        nc.sync.dma_start(out=outC_d[:, :], in_=ostC)
        nc.sync.dma_start(out=outB_d[:, :], in_=ostB)

    nc.compile()
    return nc


def _get_nc():
    if "nc" not in _CACHE:
        _CACHE["nc"] = _build()
    return _CACHE["nc"]


def _prepare(image, text, target, padding_mask):
    import ml_dtypes
    bf = ml_dtypes.bfloat16
    f8 = ml_dtypes.float8_e4m3
    image = np.asarray(image, dtype=np.float32)
    text = np.asarray(text, dtype=np.float32)
    target = np.asarray(target, dtype=np.float32)
    mask = np.asarray(padding_mask)

    keep = (mask[:, 1:] == 0)          # [B, 196] bool
    n_tokens = float(keep.sum())

    in_maps = []
    for c in range(NC_):
        sl = slice(c * BL, (c + 1) * BL)
        kb = keep[sl]                                   # [BL, 196]
        masks = np.zeros((128, MCOLS), np.float32)
        masks[:, 0:8] = kb[:, 0:C0].T                   # kA0
        masks[:68, 8:16] = kb[:, C0:TM1].T              # kA1
        kr = np.zeros((BL, T), np.float32)
        kr[:, 1:] = kb
        kr2 = kr.reshape(4, 2 * T)
        for p in range(4):
            masks[0, 16 + 2 * T * p:16 + 2 * T * (p + 1)] = kr2[p]
        in_maps.append({
            "textT": np.ascontiguousarray(
                text[sl].transpose(2, 0, 1)).astype(bf),
            "targetT": np.ascontiguousarray(
                target[sl].transpose(2, 0, 1)).astype(bf),
            "imageT": np.ascontiguousarray(
                image[sl].transpose(2, 0, 1)).astype(bf),
            "masks": masks.astype(bf),
        })
    return in_maps, n_tokens


def _combine(results, n_tokens):
    S1 = S2 = S3 = 0.0
    for r in results:
        PA = r["outA"].astype(np.float64)
        PB = r["outB"].astype(np.float64)
        PC = r["outC"].astype(np.float64)
        rows2 = [0, 64]
        S3 += (PA[:, CA_I2:CA_I2 + 4].sum() + PB[:, CB_I2_4].sum()
               + PB[:, CB_I2_5].sum())
        S2 += PA[:, CA_CLS:CA_CLS + 6].sum()
        S1 += (PC[0, CC_TSQ:CC_TSQ + 4].sum()
               + PB[:, CB_V2C0].sum() + PB[:68, CB_V2C1].sum())

    kd_tok = S1 / (n_tokens * D)
    kd_cls = S2 / (B * D)
    kd_text = (n_tokens * kd_tok + kd_cls) / (n_tokens + 1.0)
    kd_img = S3 / (B * T * D)
    return np.float32((kd_text + kd_img) / 2.0)


def kernel(image, text, target, padding_mask):
    from concourse.bass_utils import run_bass_kernel_spmd

    in_maps, n_tokens = _prepare(image, text, target, padding_mask)
    nc = _get_nc()
    results = run_bass_kernel_spmd(nc, in_maps, core_ids=list(range(NC_))).results
    return _combine(results, n_tokens)
